# revision 12
# baseline (speedup 1.0000x reference)
"""CollectAtomTriples on 8 Trainium2 NeuronCores.

For each atom a (a consecutive segment of K rows in the neighbor list),
emit all P = K*(K-1)/2 unique pairs (j < k) of its neighbor-list rows:
    idx_i_triples[a*P + p] = a
    idx_j_triples[a*P + p] = base[a] + jj[p]
    idx_k_triples[a*P + p] = base[a] + kk[p]
where base = exclusive prefix sum of per-atom counts (bincount of idx_i)
and (jj, kk) = triu_indices(K, k=1) in row-major order.

Sharding: pure data parallel over atoms — each of the 8 cores generates
the triples for n_atoms/8 consecutive atoms. Per-shard offsets are
carried in per-core input tables, so one SPMD program serves all cores.

The kernel is store-bandwidth bound: each core pushes its output slab
through the 16 SDMA engines (~27 GB/s each, ~435 GB/s/core fabric
ceiling). Three levers vs the naive version:
  * idx_i values are < n_atoms < 2^16, so the i-plane is written as
    uint16 (half the bytes); the host widens it back to int32 on the
    (free) gather path. j/k hold row indices up to n_atoms*K and stay
    int32.
  * atom-column groups ramp 1,2,4,7,... so the first stores issue
    within ~1 µs of the consts load instead of after a full 7-column
    compute, keeping the SDMA engines from idling during warmup.
  * stores are greedily balanced by byte count across the two HWDGE
    rings (sync/scalar), and pad atom-columns (beyond the shard's real
    atom count, all in the top partitions) are trimmed from the store
    APs.

Within a core, SBUF partition p owns the NA consecutive atoms
[p*NA, (p+1)*NA) of the shard (shard padded to 128*NA rows; the pad
rows are trimmed on the host). This column-major atom layout makes each
store descriptor a long contiguous run (G*P*4 bytes per partition),
which is what gets the SDMA engines near line rate.

Device kernel (per atom-column a, 128 atoms at once):
  - DVE:  out_j col = tmpl_jj + base_col[a]  (tensor_scalar add, int32)
  - ACT:  out_k col = tmpl_kk + base_col[a]  (activation Identity+bias;
          scalar operands ride the fp32 path — exact below 2^24)
  - DVE:  out_i col = zeros_u16 + atom_col[a] (tensor_scalar add, u16)
"""

import numpy as np

_BUILD_CACHE = {}


def _make_groups(N):
    """Group sizes for the N post-col0 atom-columns: a short single-
    column ramp while the pipe fills, then full 7-column groups.

    The ramp groups get uniquely-tagged SBUF tiles (no pool rotation):
    with shared tags, group n+bufs waits on group n's store COMPLETION
    (~2us HBM receipt each), which starves the DMA engines during
    warmup.
    """
    ramp = []
    rem = N
    for s in (1, 2, 4):
        if rem <= 0:
            break
        g = min(s, rem)
        ramp.append(g)
        rem -= g
    steady = []
    while rem > 0:
        g = min(7, rem)
        steady.append(g)
        rem -= g
    return ramp, steady


def _build_module(NA, P, R):
    """SPMD Bass module: 128 partitions x NA atoms each, P pairs.

    Input tables (per core, built on the host):
      cjj/ckk [128, P] i32: jj/kk template pre-offset by the partition's
              first base value (cjj[p,:] = jj + base[p*NA]).
      citm [128, P] u16: partition-start atom id, repeated P times.
      cdel [128, NA] f32-bits: base[p*NA+c] - base[p*NA] per column.
    So column 0 of each output plane equals its template row verbatim
    and is emitted as a DRAM->DRAM copy with NO compute dependency —
    issued first, it fills the ~2.5us window before the compute
    sequencers come up. Column c>=1: template + delta (i: template + c
    as an immediate).
    """
    import concourse.tile as tile
    from concourse import bacc, mybir

    dt32 = mybir.dt.int32
    du16 = mybir.dt.uint16
    # Bacc (not raw Bass): its compile() pass splits multi-sem waits into
    # EventSemaphore instructions — TRN2 instruction structs encode only
    # ONE sync-wait, and walrus rejects instructions carrying two.
    nc = bacc.Bacc()

    cjj = nc.declare_dram_parameter("cjj", [128, P], dt32, isOutput=False)
    ckk = nc.declare_dram_parameter("ckk", [128, P], dt32, isOutput=False)
    citm = nc.declare_dram_parameter("citm", [128, P], du16, isOutput=False)
    cdel = nc.declare_dram_parameter("cdel", [128, NA], dt32, isOutput=False)
    Apad = 128 * NA
    outi = nc.declare_dram_parameter("outi", [Apad, P], du16, isOutput=True)
    outj = nc.declare_dram_parameter("outj", [Apad, P], dt32, isOutput=True)
    outk = nc.declare_dram_parameter("outk", [Apad, P], dt32, isOutput=True)

    ramp, steady = _make_groups(NA - 1)
    GMAX = max(steady) if steady else 1

    def _col0(out_t):
        return out_t.rearrange("(p a) f -> p a f", a=NA)[:, 0:1, :]

    with tile.TileContext(nc) as tc:
        with (
            tc.tile_pool(name="const", bufs=1) as cpool,
            tc.tile_pool(name="work", bufs=4) as wpool,
        ):
            # Column-0 copies: no dependencies, the scalar ring starts
            # moving real output bytes while the engines finish NEFF
            # startup. All loads go on the sync ring so compute never
            # queues behind a copy.
            del_sb = cpool.tile([128, NA], dt32)
            jj_sb = cpool.tile([128, P], dt32)
            kk_sb = cpool.tile([128, P], dt32)
            it_sb = cpool.tile([128, P], du16)
            nc.sync.dma_start(out=del_sb[:], in_=cdel[:])
            nc.sync.dma_start(out=jj_sb[:], in_=cjj[:])
            nc.scalar.dma_start(out=_col0(outj), in_=cjj[:, :])
            nc.scalar.dma_start(out=_col0(outk), in_=ckk[:, :])
            nc.scalar.dma_start(out=_col0(outi), in_=citm[:, :])
            nc.sync.dma_start(out=kk_sb[:], in_=ckk[:])
            nc.sync.dma_start(out=it_sb[:], in_=citm[:])
            del_f32 = del_sb[:, :].bitcast(mybir.dt.float32)

            ring_bytes = [
                128 * (NA + P + P) * 4 + 128 * P * 2,
                128 * (P * 4 + P * 4 + P * 2),
            ]

            # NOTE on store APs: keep the partition dim an implicit full
            # `:` slice — an explicit [0:PP] changes the lowered AP so
            # the HWDGE stops spreading descriptors across the 16 SDMA
            # engines (everything lands on engine 0, ~5x slowdown).
            def _store(out_t, sb_t, c0, ncols, esize):
                dram_ap = out_t.rearrange("(p a) f -> p a f", a=NA)[
                    :, c0 : c0 + ncols, :
                ]
                sb_ap = sb_t[:, 0 : ncols * P].rearrange(
                    "p (a f) -> p a f", f=P
                )
                nbytes = 128 * ncols * P * esize
                ring = 0 if ring_bytes[0] <= ring_bytes[1] else 1
                eng = nc.sync if ring == 0 else nc.scalar
                eng.dma_start(out=dram_ap, in_=sb_ap)
                ring_bytes[ring] += nbytes

            def _j_col(tile_ap, col):
                nc.vector.tensor_scalar_add(
                    tile_ap, jj_sb[:, :], del_f32[:, col : col + 1]
                )

            def _k_col(tile_ap, col):
                nc.scalar.activation(
                    tile_ap,
                    kk_sb[:, :],
                    mybir.ActivationFunctionType.Identity,
                    bias=del_f32[:, col : col + 1],
                    scale=1.0,
                )

            def _i_col(tile_ap, col):
                # atom id = partition-start template + col (immediate).
                # Keep on DVE — gpsimd tensor ops are ~17x slower and
                # knock DVE out of its fast SBUF port mode.
                nc.vector.tensor_scalar_add(
                    tile_ap, it_sb[:, :], float(col)
                )

            a0 = 1
            for n, gmax in enumerate(ramp):
                tj = cpool.tile([128, gmax * P], dt32, tag=f"rj{n}")
                tk = cpool.tile([128, gmax * P], dt32, tag=f"rk{n}")
                ti = cpool.tile([128, gmax * P], du16, tag=f"ri{n}")
                for g in range(gmax):
                    _j_col(tj[:, g * P : (g + 1) * P], a0 + g)
                    _k_col(tk[:, g * P : (g + 1) * P], a0 + g)
                    _i_col(ti[:, g * P : (g + 1) * P], a0 + g)
                _store(outj, tj, a0, gmax, 4)
                _store(outk, tk, a0, gmax, 4)
                _store(outi, ti, a0, gmax, 2)
                a0 += gmax

            for gmax in steady:
                tj = wpool.tile([128, GMAX * P], dt32, tag="tj")
                tk = wpool.tile([128, GMAX * P], dt32, tag="tk")
                ti = wpool.tile([128, GMAX * P], du16, tag="ti")
                for g in range(gmax):
                    _j_col(tj[:, g * P : (g + 1) * P], a0 + g)
                    _k_col(tk[:, g * P : (g + 1) * P], a0 + g)
                _store(outj, tj, a0, gmax, 4)
                _store(outk, tk, a0, gmax, 4)
                # Do NOT merge i-stores across groups: bigger tiles delay
                # the store and head-of-line-block the ring FIFOs.
                for g in range(gmax):
                    _i_col(ti[:, g * P : (g + 1) * P], a0 + g)
                _store(outi, ti, a0, gmax, 2)
                a0 += gmax

    nc.finalize()
    return nc


def _get_module(NA, P, R):
    key = (NA, P, R)
    if key not in _BUILD_CACHE:
        _BUILD_CACHE[key] = _build_module(NA, P, R)
    return _BUILD_CACHE[key]


def kernel(idx_i, n_atoms, k_neighbors, _collect_timing=None):
    n_atoms = int(n_atoms)
    K = int(k_neighbors)
    P = K * (K - 1) // 2
    M = 8  # cores

    idx_i = np.asarray(idx_i, dtype=np.int32)
    counts = np.bincount(idx_i, minlength=n_atoms)[:n_atoms]
    base = (np.cumsum(counts) - counts).astype(np.int32)

    # Shard atoms: A consecutive atoms per core, padded to 128*NA so
    # every core runs the same program (pad rows trimmed after).
    A = -(-n_atoms // M)  # ceil
    NA = -(-A // 128)
    Apad = 128 * NA

    jj, kk = np.triu_indices(K, k=1)

    base_pad = np.zeros(M * Apad, dtype=np.int32)
    atom_pad = np.zeros(M * Apad, dtype=np.int32)
    for c in range(M):
        lo = c * A
        hi = min(n_atoms, lo + A)
        base_pad[c * Apad : c * Apad + (hi - lo)] = base[lo:hi]
        atom_pad[c * Apad : c * Apad + (hi - lo)] = np.arange(
            lo, hi, dtype=np.int32
        )

    in_maps = []
    for c in range(M):
        shard_base = base_pad[c * Apad : (c + 1) * Apad].reshape(128, NA)
        shard_atom = atom_pad[c * Apad : (c + 1) * Apad].reshape(128, NA)
        p0 = shard_base[:, :1]  # base at each partition's first atom
        cjj_t = jj.astype(np.int32)[None, :] + p0
        ckk_t = kk.astype(np.int32)[None, :] + p0
        citm_t = np.ascontiguousarray(
            np.broadcast_to(shard_atom[:, :1], (128, P))
        ).astype(np.uint16)
        # per-column base deltas; pad slots (filled with 0 in base_pad)
        # clamp to 0 so they stay in range (rows are trimmed anyway)
        cdel_t = np.maximum(shard_base - p0, 0).astype(np.float32)
        in_maps.append(
            {
                "cjj": cjj_t,
                "ckk": ckk_t,
                "citm": citm_t,
                "cdel": cdel_t.view(np.int32),
            }
        )
    from concourse.bass_utils import run_bass_kernel_spmd

    # All cores have the same real-atom count when M divides n_atoms;
    # otherwise the last core has fewer but runs the same (padded)
    # program, so build for the common per-core count A.
    nc = _get_module(NA, P, min(A, n_atoms))
    trace_kwargs = {}
    if _collect_timing is not None and "trace_cores" in _collect_timing:
        trace_kwargs["trace_cores"] = _collect_timing["trace_cores"]
    res = run_bass_kernel_spmd(
        nc,
        in_maps,
        list(range(M)),
        trace=_collect_timing is not None,
        **trace_kwargs,
    )
    if _collect_timing is not None:
        _collect_timing["results"] = res

    out_i = np.empty((n_atoms, P), dtype=np.int32)
    out_j = np.empty((n_atoms, P), dtype=np.int32)
    out_k = np.empty((n_atoms, P), dtype=np.int32)
    for c in range(M):
        lo = c * A
        hi = min(n_atoms, lo + A)
        out_i[lo:hi] = res.results[c]["outi"][: hi - lo]  # u16 -> i32 widen
        out_j[lo:hi] = res.results[c]["outj"][: hi - lo]
        out_k[lo:hi] = res.results[c]["outk"][: hi - lo]

    return out_i.reshape(-1), out_j.reshape(-1), out_k.reshape(-1)


# revision 13
# speedup vs baseline: 1.0830x; 1.0830x over previous
"""CollectAtomTriples on 8 Trainium2 NeuronCores.

For each atom a (a consecutive segment of K rows in the neighbor list),
emit all P = K*(K-1)/2 unique pairs (j < k) of its neighbor-list rows:
    idx_i_triples[a*P + p] = a
    idx_j_triples[a*P + p] = base[a] + jj[p]
    idx_k_triples[a*P + p] = base[a] + kk[p]
where base = exclusive prefix sum of per-atom counts (bincount of idx_i)
and (jj, kk) = triu_indices(K, k=1) in row-major order.

Sharding: pure data parallel over atoms — each of the 8 cores generates
the triples for n_atoms/8 consecutive atoms. Per-shard offsets are
carried in per-core input tables, so one SPMD program serves all cores.

The kernel is store-bandwidth bound: each core pushes its output slab
through the 16 SDMA engines (~27 GB/s each, ~435 GB/s/core fabric
ceiling — the binding limit here, NOT the nominal 358 GB/s HBM-per-NC
figure). Levers vs the naive version (104.5us -> ~90us):
  * idx_i values are < n_atoms < 2^16, so the i-plane is written as
    uint16 (half the bytes, -16.7% total); the host widens it back to
    int32 on the (free) gather path. j/k hold row indices up to
    n_atoms*K and stay int32.
  * atom-column groups ramp 1,2,4 before the steady 7-column groups so
    the first stores issue as early as possible; the ramp tiles are
    uniquely tagged (see _make_groups).
  * the consts are split into two per-ring loads (scalar cols
    duplicated in both) so each first compute op depends on exactly ONE
    load DMA — TRN2 instruction structs encode a single sync-wait.
  * stores are greedily balanced by byte count across the two HWDGE
    rings (sync/scalar).

Within a core, SBUF partition p owns the NA consecutive atoms
[p*NA, (p+1)*NA) of the shard (shard padded to 128*NA rows; the pad
rows are trimmed on the host). This column-major atom layout makes each
store descriptor a long contiguous run (G*P*4 bytes per partition),
which is what gets the SDMA engines near line rate.

Device kernel (per atom-column a, 128 atoms at once):
  - DVE:  out_j col = tmpl_jj + base_col[a]  (tensor_scalar add, int32)
  - ACT:  out_k col = tmpl_kk + base_col[a]  (activation Identity+bias;
          scalar operands ride the fp32 path — exact below 2^24)
  - DVE:  out_i col = zeros_u16 + atom_col[a] (tensor_scalar add, u16)

Hard-won scheduling facts (each cost a failed experiment, see git-less
history in the comments below):
  * store APs must keep the partition dim an implicit `:` — an explicit
    [0:128] slice makes HWDGE stop spreading descriptors across the 16
    SDMA engines (all land on engine 0, ~5x slowdown).
  * nc.gpsimd tensor ops are ~17x slower than DVE AND knock DVE off its
    fast SBUF port mode (~4x overall) — keep Q7 idle.
  * merging i-stores across groups (bigger tiles, fewer DMAs)
    head-of-line-blocks the ring FIFOs and stalls ACT via pool-buffer
    rotation: +20us.
  * a long single-column ramp (7x1) adds descriptor overhead for no
    gain; DRAM->DRAM column prefills don't help and can trigger the
    slow-engine-15 mode.
"""

import numpy as np

_BUILD_CACHE = {}


def _make_groups(NA):
    """Atom-column group sizes: small ramp-up head, small tail.

    The ramp groups get uniquely-tagged SBUF tiles (no pool rotation):
    with shared tags, group n+bufs waits on group n's store COMPLETION
    (~2us HBM receipt each), which starves the DMA engines during
    warmup.
    """
    ramp = []
    rem = NA
    for s in (1, 2, 4):
        if rem <= 0:
            break
        g = min(s, rem)
        ramp.append(g)
        rem -= g
    steady = []
    while rem > 2:
        g = min(7, rem - 2)
        steady.append(g)
        rem -= g
    if rem:
        steady.append(rem)
    return ramp, steady


def _build_module(NA, P):
    """SPMD Bass module: 128 partitions x NA atoms each, P pairs."""
    import concourse.tile as tile
    from concourse import bacc, mybir

    dt32 = mybir.dt.int32
    du16 = mybir.dt.uint16
    # Bacc (not raw Bass): its compile() pass splits multi-sem waits into
    # EventSemaphore instructions — TRN2 instruction structs encode only
    # ONE sync-wait, and walrus rejects instructions carrying two.
    nc = bacc.Bacc()

    # Two const inputs, one per HWDGE ring, each self-contained (the
    # scalar cols are duplicated in both) so every first compute op
    # depends on exactly ONE load DMA.
    #   consts0: [:, 0:P) jj row int32; [:, P:P+NA) base cols f32;
    #            [:, P+NA:P+2NA) atom-id cols f32 (bitcast in int32)
    #   consts1: same layout with the kk row.
    CW = P + 2 * NA
    consts0 = nc.declare_dram_parameter("consts0", [128, CW], dt32, isOutput=False)
    consts1 = nc.declare_dram_parameter("consts1", [128, CW], dt32, isOutput=False)
    Apad = 128 * NA
    outi = nc.declare_dram_parameter("outi", [Apad, P], du16, isOutput=True)
    outj = nc.declare_dram_parameter("outj", [Apad, P], dt32, isOutput=True)
    outk = nc.declare_dram_parameter("outk", [Apad, P], dt32, isOutput=True)

    ramp, steady = _make_groups(NA)
    GMAX = max(steady) if steady else 1

    with tile.TileContext(nc) as tc:
        with (
            tc.tile_pool(name="const", bufs=1) as cpool,
            tc.tile_pool(name="work", bufs=4) as wpool,
        ):
            c0_sb = cpool.tile([128, CW], dt32)
            c1_sb = cpool.tile([128, CW], dt32)
            nc.sync.dma_start(out=c0_sb[:], in_=consts0[:])
            nc.scalar.dma_start(out=c1_sb[:], in_=consts1[:])
            jj_sb = c0_sb[:, 0:P]
            kk_sb = c1_sb[:, 0:P]
            cols0 = c0_sb[:, P : P + 2 * NA].bitcast(mybir.dt.float32)
            cols1 = c1_sb[:, P : P + 2 * NA].bitcast(mybir.dt.float32)
            zeros_u16 = cpool.tile([128, P], du16)
            nc.vector.memset(zeros_u16[:], 0)

            ring_bytes = [128 * CW * 4, 128 * CW * 4]  # greedy balance

            # NOTE on store APs: keep the partition dim an implicit full
            # `:` slice — an explicit [0:PP] changes the lowered AP so
            # the HWDGE stops spreading descriptors across the 16 SDMA
            # engines (everything lands on engine 0, ~5x slowdown).
            def _store(out_t, sb_t, c0, ncols, esize):
                dram_ap = out_t.rearrange("(p a) f -> p a f", a=NA)[
                    :, c0 : c0 + ncols, :
                ]
                sb_ap = sb_t[:, 0 : ncols * P].rearrange(
                    "p (a f) -> p a f", f=P
                )
                nbytes = 128 * ncols * P * esize
                ring = 0 if ring_bytes[0] <= ring_bytes[1] else 1
                eng = nc.sync if ring == 0 else nc.scalar
                eng.dma_start(out=dram_ap, in_=sb_ap)
                ring_bytes[ring] += nbytes

            def _group(gmax, a0, tj, tk, ti):
                for g in range(gmax):
                    nc.vector.tensor_scalar_add(
                        tj[:, g * P : (g + 1) * P],
                        jj_sb,
                        cols0[:, a0 + g : a0 + g + 1],
                    )
                    nc.scalar.activation(
                        tk[:, g * P : (g + 1) * P],
                        kk_sb,
                        mybir.ActivationFunctionType.Identity,
                        bias=cols1[:, a0 + g : a0 + g + 1],
                        scale=1.0,
                    )
                _store(outj, tj, a0, gmax, 4)
                _store(outk, tk, a0, gmax, 4)
                # i-columns on DVE too (u16, so cheap). NOT on gpsimd
                # and NOT merged across groups — see module docstring.
                for g in range(gmax):
                    nc.vector.tensor_scalar_add(
                        ti[:, g * P : (g + 1) * P],
                        zeros_u16[:, 0:P],
                        cols0[:, NA + a0 + g : NA + a0 + g + 1],
                    )
                _store(outi, ti, a0, gmax, 2)

            a0 = 0
            for n, gmax in enumerate(ramp):
                tj = cpool.tile([128, gmax * P], dt32, tag=f"rj{n}")
                tk = cpool.tile([128, gmax * P], dt32, tag=f"rk{n}")
                ti = cpool.tile([128, gmax * P], du16, tag=f"ri{n}")
                _group(gmax, a0, tj, tk, ti)
                a0 += gmax
            for gmax in steady:
                tj = wpool.tile([128, GMAX * P], dt32, tag="tj")
                tk = wpool.tile([128, GMAX * P], dt32, tag="tk")
                ti = wpool.tile([128, GMAX * P], du16, tag="ti")
                _group(gmax, a0, tj, tk, ti)
                a0 += gmax

    nc.finalize()
    return nc


def _get_module(NA, P):
    key = (NA, P)
    if key not in _BUILD_CACHE:
        _BUILD_CACHE[key] = _build_module(NA, P)
    return _BUILD_CACHE[key]


def kernel(idx_i, n_atoms, k_neighbors, _collect_timing=None):
    n_atoms = int(n_atoms)
    K = int(k_neighbors)
    P = K * (K - 1) // 2
    M = 8  # cores

    idx_i = np.asarray(idx_i, dtype=np.int32)
    counts = np.bincount(idx_i, minlength=n_atoms)[:n_atoms]
    base = (np.cumsum(counts) - counts).astype(np.int32)

    # Shard atoms: A consecutive atoms per core, padded to 128*NA so
    # every core runs the same program (pad rows trimmed after).
    A = -(-n_atoms // M)  # ceil
    NA = -(-A // 128)
    Apad = 128 * NA

    jj, kk = np.triu_indices(K, k=1)

    base_pad = np.zeros(M * Apad, dtype=np.int32)
    atom_pad = np.zeros(M * Apad, dtype=np.int32)
    for c in range(M):
        lo = c * A
        hi = min(n_atoms, lo + A)
        base_pad[c * Apad : c * Apad + (hi - lo)] = base[lo:hi]
        atom_pad[c * Apad : c * Apad + (hi - lo)] = np.arange(
            lo, hi, dtype=np.int32
        )

    in_maps = []
    for c in range(M):
        cols = np.empty((128, 2 * NA), dtype=np.float32)
        # partition p owns shard atoms [p*NA, (p+1)*NA)
        cols[:, 0:NA] = base_pad[c * Apad : (c + 1) * Apad].reshape(128, NA)
        cols[:, NA:] = atom_pad[c * Apad : (c + 1) * Apad].reshape(128, NA)
        consts0 = np.empty((128, P + 2 * NA), dtype=np.int32)
        consts1 = np.empty((128, P + 2 * NA), dtype=np.int32)
        consts0[:, 0:P] = jj.astype(np.int32)[None, :]
        consts1[:, 0:P] = kk.astype(np.int32)[None, :]
        consts0[:, P:] = cols.view(np.int32)
        consts1[:, P:] = cols.view(np.int32)
        in_maps.append({"consts0": consts0, "consts1": consts1})

    from concourse.bass_utils import run_bass_kernel_spmd

    nc = _get_module(NA, P)
    trace_kwargs = {}
    if _collect_timing is not None and "trace_cores" in _collect_timing:
        trace_kwargs["trace_cores"] = _collect_timing["trace_cores"]
    res = run_bass_kernel_spmd(
        nc,
        in_maps,
        list(range(M)),
        trace=_collect_timing is not None,
        **trace_kwargs,
    )
    if _collect_timing is not None:
        _collect_timing["results"] = res

    out_i = np.empty((n_atoms, P), dtype=np.int32)
    out_j = np.empty((n_atoms, P), dtype=np.int32)
    out_k = np.empty((n_atoms, P), dtype=np.int32)
    for c in range(M):
        lo = c * A
        hi = min(n_atoms, lo + A)
        out_i[lo:hi] = res.results[c]["outi"][: hi - lo]  # u16 -> i32 widen
        out_j[lo:hi] = res.results[c]["outj"][: hi - lo]
        out_k[lo:hi] = res.results[c]["outk"][: hi - lo]

    return out_i.reshape(-1), out_j.reshape(-1), out_k.reshape(-1)
